# revision 9
# baseline (speedup 1.0000x reference)
"""Trainium2 Bass kernel for nn_EucCluster: pairwise Euclidean distances
x[65536,256] vs centers[1024,256] -> (argmin over points per center [1024],
min distance per point [65536], centers passthrough).

Strategy (8 NeuronCores, data-parallel over points):
  - shard x rows 8 ways (8192 points/core), replicate centers
  - per core, the PE computes v = 2*x.c - bias directly in PSUM: the xc part
    as fp32r matmuls (full PE rate) plus a rank-1 K=1 matmul appending the
    (-1) x bias row:
      A layout [n_part, m_free]: v_A = 2xc - c2  -> max over m = x2 - min_m sq
      B layout [m_part, n_free]: v_B = 2xc - x2  -> max over n = c2 - min_n sq
  - A phase: DVE reduce_max straight from PSUM -> per-point min distance.
  - B phase: ACT copies PSUM->SBUF; DVE max8 + max_index give the per-center
    max value and its first-occurrence index (exact fp32 match -> ties pick
    the lowest local index, matching jnp.argmin).
  - host combines the 8 shards: argmax over shards of the per-shard maxima
    (first-win ties = lowest global index), concatenates per-point minima.
"""

import numpy as np
import orjson

import concourse.bass as bass
import concourse.mybir as mybir
import concourse.tile as tile
import concourse.bass_utils as bass_utils
import concourse.bass2jax as bass2jax

# ---------------------------------------------------------------------------
# Walrus compat: this neuronxcc build accepts at most ONE embedded sync wait
# per BIR instruction. Tile emits several (e.g. the tile-exit drain). Rewrite
# the BIR before compile: hoist all-but-one wait of each instruction onto
# standalone single-wait EventSemaphore instructions just before it on the
# same engine (engines issue in order, so this is semantics-preserving).
# ---------------------------------------------------------------------------

_DMA_OPCODES = {
    "DMACopy", "DMA", "DmaTransposeAnt", "DMAGatherAnt", "DMAScatterAddAnt",
    "TensorLoad", "TensorSave", "KVWritebackAnt", "PagedWritebackAnt",
}


def _split_multi_waits(bir_json: bytes) -> bytes:
    j = orjson.loads(bir_json)
    n_split = 0
    for fn in j["functions"]:
        for blk in fn["blocks"]:
            out = []
            for ins in blk["instructions"]:
                si = ins.get("sync_info")
                waits = (si or {}).get("on_wait") or []
                if len(waits) > 1:
                    for k, w in enumerate(waits[:-1]):
                        n_split += 1
                        nop = {
                            "name": f"{ins['name']}-hw{k}",
                            "opcode": "EventSemaphore",
                            "engine": ins["engine"],
                            "ins": [],
                            "outs": [],
                            "sync_info": {"on_wait": [w], "on_update": []},
                        }
                        if ins.get("debug") is not None:
                            nop["debug"] = ins["debug"]
                        out.append(nop)
                    si["on_wait"] = waits[-1:]
                upds = (si or {}).get("on_update") or []
                if len(upds) > 1 and ins["opcode"] not in _DMA_OPCODES:
                    raise RuntimeError(
                        f"unsplittable multi-update on {ins['opcode']} {ins['name']}"
                    )
                out.append(ins)
            blk["instructions"] = out
    return orjson.dumps(j) if n_split else bir_json


_orig_compile_bir_kernel = bass_utils.compile_bir_kernel


def _patched_compile_bir_kernel(bir_json, tmpdir, neff_name="file.neff"):
    if isinstance(bir_json, str):
        bir_json = bir_json.encode()
    return _orig_compile_bir_kernel(
        _split_multi_waits(bir_json), tmpdir, neff_name=neff_name
    )


def _install_compat():
    bass_utils.compile_bir_kernel = _patched_compile_bir_kernel
    bass2jax.compile_bir_kernel = _patched_compile_bir_kernel


_install_compat()

# ---------------------------------------------------------------------------
# Problem constants (hardcoded per contract)
# ---------------------------------------------------------------------------

N, M, D = 65536, 1024, 256
N_CORES = 8
NLOC = N // N_CORES          # 8192 points per core
NT = NLOC // 128             # 64 point tiles of 128
MT = M // 128                # 8 center tiles of 128
NBLK = NLOC // 1024          # 8 blocks of 1024 points in the B phase
KC = D // 128                # 2 contraction chunks
F32 = mybir.dt.float32
BF16 = mybir.dt.bfloat16
U32 = mybir.dt.uint32
Act = mybir.ActivationFunctionType


def build_nc():
    nc = bass.Bass(trn_type="TRN2")

    x_in = nc.dram_tensor("xs", [NLOC, D], F32, kind="ExternalInput")
    c_in = nc.dram_tensor("centers", [M, D], F32, kind="ExternalInput")
    ident_in = nc.dram_tensor("ident", [128, 128], F32, kind="ExternalInput")

    pm_out = nc.dram_tensor("pointmin", [NLOC], F32, kind="ExternalOutput")
    ci_out = nc.dram_tensor("cand_idx", [M, 8], U32, kind="ExternalOutput")

    x2_dram = nc.dram_tensor("x2tmp", [NLOC], F32, kind="Internal")
    c2_dram = nc.dram_tensor("c2tmp", [M], F32, kind="Internal")

    with tile.TileContext(nc) as tc:
        with (
            tc.tile_pool(name="big", bufs=1) as big,
            tc.tile_pool(name="ld", bufs=3) as ld,
            tc.tile_pool(name="small", bufs=2) as small,
            tc.tile_pool(name="trash", bufs=1) as trash,
            tc.tile_pool(name="ps", bufs=2, space="PSUM") as ps,
        ):
            # persistent SBUF
            x_km = [big.tile([128, NLOC], BF16, tag=f"x_km{k}", name=f"x_km{k}")
                    for k in range(KC)]
            c2x_km = [big.tile([128, M], BF16, tag=f"c_km{k}", name=f"c_km{k}")
                      for k in range(KC)]
            vrow = big.tile([128, NLOC], F32, tag="vrow")
            ident = big.tile([128, 128], F32, tag="ident")
            x2col = big.tile([128, NT], F32, tag="x2col")
            c2col = big.tile([128, MT], F32, tag="c2col")
            pmaxcol = big.tile([128, NT], F32, tag="pmaxcol")
            x2row = big.tile([1, NLOC], F32, tag="x2row")
            c2row = big.tile([1, M], F32, tag="c2row")
            x2row_b = big.tile([1, NLOC], BF16, tag="x2row_b")
            c2row_b = big.tile([1, M], BF16, tag="c2row_b")
            negones_b = big.tile([1, 128], BF16, tag="negones_b")

            nc.sync.dma_start(out=ident[:], in_=ident_in[:])
            nc.vector.memset(negones_b[:], -1.0)

            # ---------------- centers ingest ----------------
            # natural [m,256] tiles -> square-accum (c2) + PE transpose -> 2*cT
            sq_trash = trash.tile([128, D], F32, tag="sq_trash")
            for r in range(MT):
                c_nat = ld.tile([128, D], F32, tag="c_nat")
                nc.sync.dma_start(out=c_nat[:], in_=c_in[r * 128:(r + 1) * 128, :])
                nc.scalar.activation(
                    sq_trash[:], c_nat[:], Act.Square,
                    accum_out=c2col[:, r:r + 1],
                )
                for k in range(KC):
                    pT = ps.tile([128, 128], F32, tag="psT")
                    nc.tensor.transpose(
                        pT[:], c_nat[:, k * 128:(k + 1) * 128], ident[:]
                    )
                    nc.scalar.mul(
                        c2x_km[k][:, r * 128:(r + 1) * 128], pT[:], 2.0
                    )

            # c2 column -> DRAM in center order -> [1, M] row -> fp32r round
            nc.sync.dma_start(
                out=c2_dram[:].rearrange("(r p) -> p r", p=128), in_=c2col[:]
            )
            nc.sync.dma_start(out=c2row[:], in_=c2_dram[:].rearrange("(o m) -> o m", o=1))
            nc.scalar.copy(c2row_b[:], c2row[:])

            # ---------------- x ingest ----------------
            XB = 8  # tiles per load batch
            for b in range(NT // XB):
                x_nat = ld.tile([128, XB, D], F32, tag="x_nat")
                nc.sync.dma_start(
                    out=x_nat[:],
                    in_=x_in[b * XB * 128:(b + 1) * XB * 128, :].rearrange(
                        "(t p) d -> p t d", p=128
                    ),
                )
                for tl in range(XB):
                    t = b * XB + tl
                    nc.scalar.activation(
                        sq_trash[:], x_nat[:, tl, :], Act.Square,
                        accum_out=x2col[:, t:t + 1],
                    )
                    for k in range(KC):
                        pT = ps.tile([128, 128], F32, tag="psT")
                        nc.tensor.transpose(
                            pT[:], x_nat[:, tl, k * 128:(k + 1) * 128], ident[:]
                        )
                        nc.scalar.copy(x_km[k][:, t * 128:(t + 1) * 128], pT[:])

            nc.sync.dma_start(
                out=x2_dram[:].rearrange("(t p) -> p t", p=128), in_=x2col[:]
            )
            nc.sync.dma_start(out=x2row[:], in_=x2_dram[:].rearrange("(o n) -> o n", o=1))
            nc.scalar.copy(x2row_b[:], x2row[:])

            # ---------------- A phase: per-point min over centers ----------
            for t in range(NT):
                psA = ps.tile([128, M], F32, tag="psA")
                for mc in range(M // 512):
                    reg = psA[:, mc * 512:(mc + 1) * 512]
                    msl = slice(mc * 512, (mc + 1) * 512)
                    for k in range(KC):
                        nc.tensor.matmul(
                            reg,
                            lhsT=x_km[k][:, t * 128:(t + 1) * 128],
                            rhs=c2x_km[k][:, msl],
                            start=(k == 0),
                            stop=False,
                        )
                    nc.tensor.matmul(
                        reg,
                        lhsT=negones_b[:],
                        rhs=c2row_b[:, msl],
                        start=False,
                        stop=True,
                    )
                nc.vector.tensor_reduce(
                    out=pmaxcol[:, t:t + 1], in_=psA[:],
                    axis=mybir.AxisListType.X, op=mybir.AluOpType.max,
                )

            # pointmin = sqrt(max(x2 - pmax, 0)), written in point order
            sqmin = small.tile([128, NT], F32, tag="sqmin")
            nc.vector.tensor_sub(sqmin[:], x2col[:], pmaxcol[:])
            nc.vector.tensor_scalar_max(sqmin[:], sqmin[:], 0.0)
            nc.scalar.sqrt(sqmin[:], sqmin[:])
            nc.sync.dma_start(
                out=pm_out[:].rearrange("(t p) -> p t", p=128), in_=sqmin[:]
            )

            # ---------------- B phase: per-center argmin over points ------
            for r in range(MT):
                for nb in range(NBLK):
                    psB = ps.tile([128, 1024], F32, tag="psA")
                    for nh in range(2):
                        reg = psB[:, nh * 512:(nh + 1) * 512]
                        nsl = slice(nb * 1024 + nh * 512, nb * 1024 + (nh + 1) * 512)
                        for k in range(KC):
                            nc.tensor.matmul(
                                reg,
                                lhsT=c2x_km[k][:, r * 128:(r + 1) * 128],
                                rhs=x_km[k][:, nsl],
                                start=(k == 0),
                                stop=False,
                            )
                        nc.tensor.matmul(
                            reg,
                            lhsT=negones_b[:],
                            rhs=x2row_b[:, nsl],
                            start=False,
                            stop=True,
                        )
                    nc.scalar.copy(vrow[:, nb * 1024:(nb + 1) * 1024], psB[:])
                top8 = small.tile([128, 8], F32, tag="top8")
                idx8 = small.tile([128, 8], U32, tag="idx8")
                nc.vector.max(top8[:], vrow[:])
                nc.vector.max_index(idx8[:], top8[:], vrow[:])
                nc.sync.dma_start(
                    out=ci_out[r * 128:(r + 1) * 128, :], in_=idx8[:]
                )

    return nc


_CACHED = {}


def _get_nc():
    if "nc" not in _CACHED:
        _CACHED["nc"] = build_nc()
    return _CACHED["nc"]


def kernel(x: np.ndarray, centers: np.ndarray):
    from concourse.bass_utils import run_bass_kernel_spmd

    x = np.ascontiguousarray(np.asarray(x, dtype=np.float32))
    centers = np.ascontiguousarray(np.asarray(centers, dtype=np.float32))
    assert x.shape == (N, D) and centers.shape == (M, D)

    nc = _get_nc()
    ident = np.eye(128, dtype=np.float32)
    in_maps = [
        {
            "xs": x[c * NLOC:(c + 1) * NLOC],
            "centers": centers,
            "ident": ident,
        }
        for c in range(N_CORES)
    ]
    res = run_bass_kernel_spmd(nc, in_maps, core_ids=list(range(N_CORES)))

    pointmin = np.concatenate([res.results[c]["pointmin"] for c in range(N_CORES)])

    # candidates: [M, N_CORES*8] global point indices (device gives local
    # top-8 per shard; exact winner decided here in float64, ties -> lowest
    # index, matching jnp.argmin)
    cand = np.concatenate(
        [
            res.results[c]["cand_idx"].astype(np.int64) + c * NLOC
            for c in range(N_CORES)
        ],
        axis=1,
    )                                                                      # [M, 64]
    valid = (cand >= 0) & (cand < N)
    cand_safe = np.where(valid, cand, 0)
    x64 = x.astype(np.float64)
    c64 = centers.astype(np.float64)
    xg = x64[cand_safe]                                                    # [M, 64, D]
    sq = ((xg - c64[:, None, :]) ** 2).sum(-1)                             # [M, 64]
    sq[~valid] = np.inf
    best = sq.min(axis=1, keepdims=True)
    pick = np.where(sq == best, cand_safe, np.iinfo(np.int64).max)
    argmin_idx = pick.min(axis=1).astype(np.int32)
    return argmin_idx, pointmin.astype(np.float32), centers


# revision 10
# speedup vs baseline: 1.7109x; 1.7109x over previous
"""Trainium2 Bass kernel for nn_EucCluster: pairwise Euclidean distances
x[65536,256] vs centers[1024,256] -> (argmin over points per center [1024],
min distance per point [65536], centers passthrough).

Strategy (8 NeuronCores, data-parallel over points):
  - shard x rows 8 ways (8192 points/core), replicate centers
  - per core, the PE computes v = 2*x.c - bias directly in PSUM: the xc part
    as fp32r matmuls (full PE rate) plus a rank-1 K=1 matmul appending the
    (-1) x bias row:
      A layout [n_part, m_free]: v_A = 2xc - c2  -> max over m = x2 - min_m sq
      B layout [m_part, n_free]: v_B = 2xc - x2  -> max over n = c2 - min_n sq
  - A phase: DVE reduce_max straight from PSUM -> per-point min distance.
  - B phase: ACT copies PSUM->SBUF; DVE max8 + max_index give the per-center
    max value and its first-occurrence index (exact fp32 match -> ties pick
    the lowest local index, matching jnp.argmin).
  - host combines the 8 shards: argmax over shards of the per-shard maxima
    (first-win ties = lowest global index), concatenates per-point minima.
"""

import numpy as np
import orjson

import concourse.bass as bass
import concourse.mybir as mybir
import concourse.tile as tile
import concourse.bass_utils as bass_utils
import concourse.bass2jax as bass2jax

# ---------------------------------------------------------------------------
# Walrus compat: this neuronxcc build accepts at most ONE embedded sync wait
# per BIR instruction. Tile emits several (e.g. the tile-exit drain). Rewrite
# the BIR before compile: hoist all-but-one wait of each instruction onto
# standalone single-wait EventSemaphore instructions just before it on the
# same engine (engines issue in order, so this is semantics-preserving).
# ---------------------------------------------------------------------------

_DMA_OPCODES = {
    "DMACopy", "DMA", "DmaTransposeAnt", "DMAGatherAnt", "DMAScatterAddAnt",
    "TensorLoad", "TensorSave", "KVWritebackAnt", "PagedWritebackAnt",
}


def _split_multi_waits(bir_json: bytes) -> bytes:
    j = orjson.loads(bir_json)
    n_split = 0
    for fn in j["functions"]:
        for blk in fn["blocks"]:
            out = []
            for ins in blk["instructions"]:
                si = ins.get("sync_info")
                waits = (si or {}).get("on_wait") or []
                if len(waits) > 1:
                    for k, w in enumerate(waits[:-1]):
                        n_split += 1
                        nop = {
                            "name": f"{ins['name']}-hw{k}",
                            "opcode": "EventSemaphore",
                            "engine": ins["engine"],
                            "ins": [],
                            "outs": [],
                            "sync_info": {"on_wait": [w], "on_update": []},
                        }
                        if ins.get("debug") is not None:
                            nop["debug"] = ins["debug"]
                        out.append(nop)
                    si["on_wait"] = waits[-1:]
                upds = (si or {}).get("on_update") or []
                if len(upds) > 1 and ins["opcode"] not in _DMA_OPCODES:
                    raise RuntimeError(
                        f"unsplittable multi-update on {ins['opcode']} {ins['name']}"
                    )
                out.append(ins)
            blk["instructions"] = out
    return orjson.dumps(j) if n_split else bir_json


_orig_compile_bir_kernel = bass_utils.compile_bir_kernel


def _patched_compile_bir_kernel(bir_json, tmpdir, neff_name="file.neff"):
    if isinstance(bir_json, str):
        bir_json = bir_json.encode()
    return _orig_compile_bir_kernel(
        _split_multi_waits(bir_json), tmpdir, neff_name=neff_name
    )


def _install_compat():
    bass_utils.compile_bir_kernel = _patched_compile_bir_kernel
    bass2jax.compile_bir_kernel = _patched_compile_bir_kernel


_install_compat()

# ---------------------------------------------------------------------------
# Problem constants (hardcoded per contract)
# ---------------------------------------------------------------------------

N, M, D = 65536, 1024, 256
N_CORES = 8
NLOC = N // N_CORES          # 8192 points per core
NT = NLOC // 128             # 64 point tiles of 128
MT = M // 128                # 8 center tiles of 128
NBLK = NLOC // 1024          # 8 blocks of 1024 points in the B phase
KC = D // 128                # 2 contraction chunks
F32 = mybir.dt.float32
BF16 = mybir.dt.bfloat16
U32 = mybir.dt.uint32
Act = mybir.ActivationFunctionType


def build_nc():
    nc = bass.Bass(trn_type="TRN2")

    x_in = nc.dram_tensor("xs", [NLOC, D], F32, kind="ExternalInput")
    c_in = nc.dram_tensor("centers", [M, D], F32, kind="ExternalInput")
    ident_in = nc.dram_tensor("ident", [128, 128], F32, kind="ExternalInput")

    pm_out = nc.dram_tensor("pointmin", [NLOC], F32, kind="ExternalOutput")
    ci_out = nc.dram_tensor("cand_idx", [M, 8], U32, kind="ExternalOutput")

    x2_dram = nc.dram_tensor("x2tmp", [NLOC], F32, kind="Internal")
    c2_dram = nc.dram_tensor("c2tmp", [M], F32, kind="Internal")

    with tile.TileContext(nc) as tc:
        with (
            tc.tile_pool(name="big", bufs=1) as big,
            tc.tile_pool(name="ld", bufs=3) as ld,
            tc.tile_pool(name="small", bufs=2) as small,
            tc.tile_pool(name="trash", bufs=1) as trash,
            tc.tile_pool(name="ps", bufs=2, space="PSUM") as ps,
        ):
            # persistent SBUF
            x_km = [big.tile([128, NLOC], BF16, tag=f"x_km{k}", name=f"x_km{k}")
                    for k in range(KC)]
            c2x_km = [big.tile([128, M], BF16, tag=f"c_km{k}", name=f"c_km{k}")
                      for k in range(KC)]
            ident = big.tile([128, 128], F32, tag="ident")
            x2col = big.tile([128, NT], F32, tag="x2col")
            c2col = big.tile([128, MT], F32, tag="c2col")
            pmaxcol = big.tile([128, NT], F32, tag="pmaxcol")
            x2rep_b = big.tile([128, NLOC], BF16, tag="x2rep_b")
            c2rep_b = big.tile([128, M], BF16, tag="c2rep_b")
            unif = big.tile([128, 128], BF16, tag="unif")

            nc.sync.dma_start(out=ident[:], in_=ident_in[:])
            nc.vector.memset(unif[:], -1.0 / 128.0)

            # ---------------- centers ingest ----------------
            # natural [m,256] tiles -> square-accum (c2) + PE transpose -> 2*cT
            sq_trash = trash.tile([128, D], F32, tag="sq_trash")
            for r in range(MT):
                c_nat = ld.tile([128, D], F32, tag="c_nat")
                nc.sync.dma_start(out=c_nat[:], in_=c_in[r * 128:(r + 1) * 128, :])
                nc.scalar.activation(
                    sq_trash[:], c_nat[:], Act.Square,
                    accum_out=c2col[:, r:r + 1],
                )
                for k in range(KC):
                    pT = ps.tile([128, 128], F32, tag="psB")
                    nc.tensor.transpose(
                        pT[:], c_nat[:, k * 128:(k + 1) * 128], ident[:]
                    )
                    nc.scalar.mul(
                        c2x_km[k][:, r * 128:(r + 1) * 128], pT[:], 2.0
                    )

            # c2 column -> DRAM in center order -> broadcast bf16 rep tile
            nc.sync.dma_start(
                out=c2_dram[:].rearrange("(r p) -> p r", p=128), in_=c2col[:]
            )
            c2_bcast = bass.AP(tensor=c2_dram, offset=0, ap=[[0, 128], [1, M]])
            nc.gpsimd.dma_start(out=c2rep_b[:], in_=c2_bcast)

            # ---------------- x ingest ----------------
            XB = 8  # tiles per load batch
            for b in range(NT // XB):
                x_nat = ld.tile([128, XB, D], F32, tag="x_nat")
                nc.sync.dma_start(
                    out=x_nat[:],
                    in_=x_in[b * XB * 128:(b + 1) * XB * 128, :].rearrange(
                        "(t p) d -> p t d", p=128
                    ),
                )
                for tl in range(XB):
                    t = b * XB + tl
                    nc.scalar.activation(
                        sq_trash[:], x_nat[:, tl, :], Act.Square,
                        accum_out=x2col[:, t:t + 1],
                    )
                    for k in range(KC):
                        pT = ps.tile([128, 128], F32, tag="psB")
                        nc.tensor.transpose(
                            pT[:], x_nat[:, tl, k * 128:(k + 1) * 128], ident[:]
                        )
                        nc.scalar.copy(x_km[k][:, t * 128:(t + 1) * 128], pT[:])

            nc.sync.dma_start(
                out=x2_dram[:].rearrange("(t p) -> p t", p=128), in_=x2col[:]
            )
            x2_bcast = bass.AP(tensor=x2_dram, offset=0, ap=[[0, 128], [1, NLOC]])
            nc.gpsimd.dma_start(out=x2rep_b[:], in_=x2_bcast)

            # ---------------- A phase: per-point min over centers ----------
            for t in range(NT):
                psA = ps.tile([128, M], F32, tag="psA")
                for mc in range(M // 512):
                    reg = psA[:, mc * 512:(mc + 1) * 512]
                    msl = slice(mc * 512, (mc + 1) * 512)
                    for k in range(KC):
                        nc.tensor.matmul(
                            reg,
                            lhsT=x_km[k][:, t * 128:(t + 1) * 128],
                            rhs=c2x_km[k][:, msl],
                            start=(k == 0),
                            stop=False,
                        )
                    nc.tensor.matmul(
                        reg,
                        lhsT=unif[:],
                        rhs=c2rep_b[:, msl],
                        start=False,
                        stop=True,
                    )
                nc.vector.tensor_reduce(
                    out=pmaxcol[:, t:t + 1], in_=psA[:],
                    axis=mybir.AxisListType.X, op=mybir.AluOpType.max,
                )

            # pointmin = sqrt(max(x2 - pmax, 0)), written in point order
            sqmin = small.tile([128, NT], F32, tag="sqmin")
            nc.vector.tensor_sub(sqmin[:], x2col[:], pmaxcol[:])
            nc.vector.tensor_scalar_max(sqmin[:], sqmin[:], 0.0)
            nc.scalar.sqrt(sqmin[:], sqmin[:])
            nc.sync.dma_start(
                out=pm_out[:].rearrange("(t p) -> p t", p=128), in_=sqmin[:]
            )

            # ---------------- B phase: per-center argmin over points ------
            for r in range(MT):
                vrow = small.tile([128, NLOC], F32, tag="vrow", name=f"vrow{r}")
                for nb in range(NBLK):
                    psB = ps.tile([128, 1024], F32, tag="psB")
                    for nh in range(2):
                        reg = psB[:, nh * 512:(nh + 1) * 512]
                        nsl = slice(nb * 1024 + nh * 512, nb * 1024 + (nh + 1) * 512)
                        for k in range(KC):
                            nc.tensor.matmul(
                                reg,
                                lhsT=c2x_km[k][:, r * 128:(r + 1) * 128],
                                rhs=x_km[k][:, nsl],
                                start=(k == 0),
                                stop=False,
                            )
                        nc.tensor.matmul(
                            reg,
                            lhsT=unif[:],
                            rhs=x2rep_b[:, nsl],
                            start=False,
                            stop=True,
                        )
                    nc.scalar.copy(vrow[:, nb * 1024:(nb + 1) * 1024], psB[:])
                top8 = small.tile([128, 8], F32, tag="top8")
                idx8 = small.tile([128, 8], U32, tag="idx8")
                nc.vector.max(top8[:], vrow[:])
                nc.vector.max_index(idx8[:], top8[:], vrow[:])
                nc.sync.dma_start(
                    out=ci_out[r * 128:(r + 1) * 128, :], in_=idx8[:]
                )

    return nc


_CACHED = {}


def _get_nc():
    if "nc" not in _CACHED:
        _CACHED["nc"] = build_nc()
    return _CACHED["nc"]


def kernel(x: np.ndarray, centers: np.ndarray):
    from concourse.bass_utils import run_bass_kernel_spmd

    x = np.ascontiguousarray(np.asarray(x, dtype=np.float32))
    centers = np.ascontiguousarray(np.asarray(centers, dtype=np.float32))
    assert x.shape == (N, D) and centers.shape == (M, D)

    nc = _get_nc()
    ident = np.eye(128, dtype=np.float32)
    in_maps = [
        {
            "xs": x[c * NLOC:(c + 1) * NLOC],
            "centers": centers,
            "ident": ident,
        }
        for c in range(N_CORES)
    ]
    res = run_bass_kernel_spmd(nc, in_maps, core_ids=list(range(N_CORES)))

    pointmin = np.concatenate([res.results[c]["pointmin"] for c in range(N_CORES)])

    # candidates: [M, N_CORES*8] global point indices (device gives local
    # top-8 per shard; exact winner decided here in float64, ties -> lowest
    # index, matching jnp.argmin)
    cand = np.concatenate(
        [
            res.results[c]["cand_idx"].astype(np.int64) + c * NLOC
            for c in range(N_CORES)
        ],
        axis=1,
    )                                                                      # [M, 64]
    valid = (cand >= 0) & (cand < N)
    cand_safe = np.where(valid, cand, 0)
    x64 = x.astype(np.float64)
    c64 = centers.astype(np.float64)
    xg = x64[cand_safe]                                                    # [M, 64, D]
    sq = ((xg - c64[:, None, :]) ** 2).sum(-1)                             # [M, 64]
    sq[~valid] = np.inf
    best = sq.min(axis=1, keepdims=True)
    pick = np.where(sq == best, cand_safe, np.iinfo(np.int64).max)
    argmin_idx = pick.min(axis=1).astype(np.int32)
    return argmin_idx, pointmin.astype(np.float32), centers


# revision 12
# speedup vs baseline: 1.7357x; 1.0145x over previous
"""Trainium2 Bass kernel for nn_EucCluster: pairwise Euclidean distances
x[65536,256] vs centers[1024,256] -> (argmin over points per center [1024],
min distance per point [65536], centers passthrough).

Strategy (8 NeuronCores, data-parallel over points):
  - shard x rows 8 ways (8192 points/core), replicate centers
  - per core, the PE computes v = 2*x.c - bias directly in PSUM: the xc part
    as fp32r matmuls (full PE rate) plus a rank-1 K=1 matmul appending the
    (-1) x bias row:
      A layout [n_part, m_free]: v_A = 2xc - c2  -> max over m = x2 - min_m sq
      B layout [m_part, n_free]: v_B = 2xc - x2  -> max over n = c2 - min_n sq
  - A phase: DVE reduce_max straight from PSUM -> per-point min distance.
  - B phase: ACT copies PSUM->SBUF; DVE max8 + max_index give the per-center
    max value and its first-occurrence index (exact fp32 match -> ties pick
    the lowest local index, matching jnp.argmin).
  - host combines the 8 shards: argmax over shards of the per-shard maxima
    (first-win ties = lowest global index), concatenates per-point minima.
"""

import numpy as np
import orjson

import concourse.bass as bass
import concourse.mybir as mybir
import concourse.tile as tile
import concourse.bass_utils as bass_utils
import concourse.bass2jax as bass2jax

# ---------------------------------------------------------------------------
# Walrus compat: this neuronxcc build accepts at most ONE embedded sync wait
# per BIR instruction. Tile emits several (e.g. the tile-exit drain). Rewrite
# the BIR before compile: hoist all-but-one wait of each instruction onto
# standalone single-wait EventSemaphore instructions just before it on the
# same engine (engines issue in order, so this is semantics-preserving).
# ---------------------------------------------------------------------------

_DMA_OPCODES = {
    "DMACopy", "DMA", "DmaTransposeAnt", "DMAGatherAnt", "DMAScatterAddAnt",
    "TensorLoad", "TensorSave", "KVWritebackAnt", "PagedWritebackAnt",
}


def _split_multi_waits(bir_json: bytes) -> bytes:
    j = orjson.loads(bir_json)
    n_split = 0
    for fn in j["functions"]:
        for blk in fn["blocks"]:
            out = []
            for ins in blk["instructions"]:
                si = ins.get("sync_info")
                waits = (si or {}).get("on_wait") or []
                if len(waits) > 1:
                    for k, w in enumerate(waits[:-1]):
                        n_split += 1
                        nop = {
                            "name": f"{ins['name']}-hw{k}",
                            "opcode": "EventSemaphore",
                            "engine": ins["engine"],
                            "ins": [],
                            "outs": [],
                            "sync_info": {"on_wait": [w], "on_update": []},
                        }
                        if ins.get("debug") is not None:
                            nop["debug"] = ins["debug"]
                        out.append(nop)
                    si["on_wait"] = waits[-1:]
                upds = (si or {}).get("on_update") or []
                if len(upds) > 1 and ins["opcode"] not in _DMA_OPCODES:
                    raise RuntimeError(
                        f"unsplittable multi-update on {ins['opcode']} {ins['name']}"
                    )
                out.append(ins)
            blk["instructions"] = out
    return orjson.dumps(j) if n_split else bir_json


_orig_compile_bir_kernel = bass_utils.compile_bir_kernel


def _patched_compile_bir_kernel(bir_json, tmpdir, neff_name="file.neff"):
    if isinstance(bir_json, str):
        bir_json = bir_json.encode()
    return _orig_compile_bir_kernel(
        _split_multi_waits(bir_json), tmpdir, neff_name=neff_name
    )


def _install_compat():
    bass_utils.compile_bir_kernel = _patched_compile_bir_kernel
    bass2jax.compile_bir_kernel = _patched_compile_bir_kernel


_install_compat()

# ---------------------------------------------------------------------------
# Problem constants (hardcoded per contract)
# ---------------------------------------------------------------------------

N, M, D = 65536, 1024, 256
N_CORES = 8
NLOC = N // N_CORES          # 8192 points per core
NT = NLOC // 128             # 64 point tiles of 128
MT = M // 128                # 8 center tiles of 128
NBLK = NLOC // 1024          # 8 blocks of 1024 points in the B phase
KC = D // 128                # 2 contraction chunks
F32 = mybir.dt.float32
BF16 = mybir.dt.bfloat16
U32 = mybir.dt.uint32
Act = mybir.ActivationFunctionType


def build_nc():
    nc = bass.Bass(trn_type="TRN2")

    x_in = nc.dram_tensor("xs", [NLOC, D], F32, kind="ExternalInput")
    c_in = nc.dram_tensor("centers", [M, D], F32, kind="ExternalInput")
    ident_in = nc.dram_tensor("ident", [128, 128], F32, kind="ExternalInput")

    pm_out = nc.dram_tensor("pointmin", [NLOC], F32, kind="ExternalOutput")
    ci_out = nc.dram_tensor("cand_idx", [M, 8], U32, kind="ExternalOutput")

    x2_dram = nc.dram_tensor("x2tmp", [NLOC], F32, kind="Internal")
    c2_dram = nc.dram_tensor("c2tmp", [M], F32, kind="Internal")

    with tile.TileContext(nc) as tc:
        with (
            tc.tile_pool(name="big", bufs=1) as big,
            tc.tile_pool(name="ld", bufs=3) as ld,
            tc.tile_pool(name="small", bufs=2) as small,
            tc.tile_pool(name="trash", bufs=1) as trash,
            tc.tile_pool(name="ps", bufs=2, space="PSUM") as ps,
        ):
            # persistent SBUF
            x_km = [big.tile([128, NLOC], BF16, tag=f"x_km{k}", name=f"x_km{k}")
                    for k in range(KC)]
            c2x_km = [big.tile([128, M], BF16, tag=f"c_km{k}", name=f"c_km{k}")
                      for k in range(KC)]
            ident = big.tile([128, 128], F32, tag="ident")
            x2col = big.tile([128, NT], F32, tag="x2col")
            c2col = big.tile([128, MT], F32, tag="c2col")
            pmaxcol = big.tile([128, NT], F32, tag="pmaxcol")
            x2rep_b = big.tile([128, NLOC], BF16, tag="x2rep_b")
            c2rep_b = big.tile([128, M], BF16, tag="c2rep_b")
            unif = big.tile([128, 128], BF16, tag="unif")

            nc.sync.dma_start(out=ident[:], in_=ident_in[:])
            nc.vector.memset(unif[:], -1.0 / 128.0)

            # ---------------- centers ingest ----------------
            # natural [m,256] tiles -> square-accum (c2) + PE transpose -> 2*cT
            sq_trash = trash.tile([128, D], F32, tag="sq_trash")
            for r in range(MT):
                c_nat = ld.tile([128, D], F32, tag="c_nat")
                nc.sync.dma_start(out=c_nat[:], in_=c_in[r * 128:(r + 1) * 128, :])
                nc.scalar.activation(
                    sq_trash[:], c_nat[:], Act.Square,
                    accum_out=c2col[:, r:r + 1],
                )
                for k in range(KC):
                    pT = ps.tile([128, 128], F32, tag="psB")
                    nc.tensor.transpose(
                        pT[:], c_nat[:, k * 128:(k + 1) * 128], ident[:]
                    )
                    nc.scalar.mul(
                        c2x_km[k][:, r * 128:(r + 1) * 128], pT[:], 2.0
                    )

            # c2 column -> DRAM in center order -> broadcast bf16 rep tile
            nc.sync.dma_start(
                out=c2_dram[:].rearrange("(r p) -> p r", p=128), in_=c2col[:]
            )
            c2_bcast = bass.AP(tensor=c2_dram, offset=0, ap=[[0, 128], [1, M]])
            nc.gpsimd.dma_start(out=c2rep_b[:], in_=c2_bcast)

            # ---------------- x ingest ----------------
            XB = 8  # tiles per load batch
            for b in range(NT // XB):
                x_nat = ld.tile([128, XB, D], F32, tag="x_nat")
                nc.sync.dma_start(
                    out=x_nat[:],
                    in_=x_in[:].rearrange("(p t) d -> p t d", t=NT)[
                        :, b * XB:(b + 1) * XB, :
                    ],
                )
                for tl in range(XB):
                    t = b * XB + tl
                    nc.scalar.activation(
                        sq_trash[:], x_nat[:, tl, :], Act.Square,
                        accum_out=x2col[:, t:t + 1],
                    )
                    for k in range(KC):
                        pT = ps.tile([128, 128], F32, tag="psB")
                        nc.tensor.transpose(
                            pT[:], x_nat[:, tl, k * 128:(k + 1) * 128], ident[:]
                        )
                        nc.scalar.copy(x_km[k][:, t * 128:(t + 1) * 128], pT[:])

            nc.sync.dma_start(
                out=x2_dram[:].rearrange("(t p) -> p t", p=128), in_=x2col[:]
            )
            x2_bcast = bass.AP(tensor=x2_dram, offset=0, ap=[[0, 128], [1, NLOC]])
            nc.gpsimd.dma_start(out=x2rep_b[:], in_=x2_bcast)

            # ---------------- A phase: per-point min over centers ----------
            for t in range(NT):
                psA = ps.tile([128, M], F32, tag="psA")
                for mc in range(M // 512):
                    reg = psA[:, mc * 512:(mc + 1) * 512]
                    msl = slice(mc * 512, (mc + 1) * 512)
                    for k in range(KC):
                        nc.tensor.matmul(
                            reg,
                            lhsT=x_km[k][:, t * 128:(t + 1) * 128],
                            rhs=c2x_km[k][:, msl],
                            start=(k == 0),
                            stop=False,
                        )
                    nc.tensor.matmul(
                        reg,
                        lhsT=unif[:],
                        rhs=c2rep_b[:, msl],
                        start=False,
                        stop=True,
                    )
                nc.vector.tensor_reduce(
                    out=pmaxcol[:, t:t + 1], in_=psA[:],
                    axis=mybir.AxisListType.X, op=mybir.AluOpType.max,
                )

            # pointmin = sqrt(max(x2 - pmax, 0)), written in point order
            sqmin = small.tile([128, NT], F32, tag="sqmin")
            nc.vector.tensor_sub(sqmin[:], x2col[:], pmaxcol[:])
            nc.vector.tensor_scalar_max(sqmin[:], sqmin[:], 0.0)
            nc.scalar.sqrt(sqmin[:], sqmin[:])
            nc.sync.dma_start(
                out=pm_out[:].rearrange("(p t) -> p t", p=128), in_=sqmin[:]
            )

            # ---------------- B phase: per-center argmin over points ------
            for r in range(MT):
                vrow = small.tile([128, NLOC], F32, tag="vrow", name=f"vrow{r}")
                for nb in range(NBLK):
                    psB = ps.tile([128, 1024], F32, tag="psB")
                    for nh in range(2):
                        reg = psB[:, nh * 512:(nh + 1) * 512]
                        nsl = slice(nb * 1024 + nh * 512, nb * 1024 + (nh + 1) * 512)
                        for k in range(KC):
                            nc.tensor.matmul(
                                reg,
                                lhsT=c2x_km[k][:, r * 128:(r + 1) * 128],
                                rhs=x_km[k][:, nsl],
                                start=(k == 0),
                                stop=False,
                            )
                        nc.tensor.matmul(
                            reg,
                            lhsT=unif[:],
                            rhs=x2rep_b[:, nsl],
                            start=False,
                            stop=True,
                        )
                    nc.scalar.copy(vrow[:, nb * 1024:(nb + 1) * 1024], psB[:])
                top8 = small.tile([128, 8], F32, tag="top8")
                idx8 = small.tile([128, 8], U32, tag="idx8")
                nc.vector.max(top8[:], vrow[:])
                nc.vector.max_index(idx8[:], top8[:], vrow[:])
                nc.sync.dma_start(
                    out=ci_out[r * 128:(r + 1) * 128, :], in_=idx8[:]
                )

    return nc


_CACHED = {}


def _get_nc():
    if "nc" not in _CACHED:
        _CACHED["nc"] = build_nc()
    return _CACHED["nc"]


def kernel(x: np.ndarray, centers: np.ndarray):
    from concourse.bass_utils import run_bass_kernel_spmd

    x = np.ascontiguousarray(np.asarray(x, dtype=np.float32))
    centers = np.ascontiguousarray(np.asarray(centers, dtype=np.float32))
    assert x.shape == (N, D) and centers.shape == (M, D)

    nc = _get_nc()
    ident = np.eye(128, dtype=np.float32)
    in_maps = [
        {
            "xs": x[c * NLOC:(c + 1) * NLOC],
            "centers": centers,
            "ident": ident,
        }
        for c in range(N_CORES)
    ]
    res = run_bass_kernel_spmd(nc, in_maps, core_ids=list(range(N_CORES)))

    pointmin = np.concatenate([res.results[c]["pointmin"] for c in range(N_CORES)])

    # candidates: [M, N_CORES*8] global point indices (device gives local
    # top-8 per shard; exact winner decided here in float64, ties -> lowest
    # index, matching jnp.argmin)
    def col_to_point(col):
        return (col % 128) * NT + (col // 128)

    cand = np.concatenate(
        [
            col_to_point(res.results[c]["cand_idx"].astype(np.int64)) + c * NLOC
            for c in range(N_CORES)
        ],
        axis=1,
    )                                                                      # [M, 64]
    valid = (cand >= 0) & (cand < N)
    cand_safe = np.where(valid, cand, 0)
    x64 = x.astype(np.float64)
    c64 = centers.astype(np.float64)
    xg = x64[cand_safe]                                                    # [M, 64, D]
    sq = ((xg - c64[:, None, :]) ** 2).sum(-1)                             # [M, 64]
    sq[~valid] = np.inf
    best = sq.min(axis=1, keepdims=True)
    pick = np.where(sq == best, cand_safe, np.iinfo(np.int64).max)
    argmin_idx = pick.min(axis=1).astype(np.int32)
    return argmin_idx, pointmin.astype(np.float32), centers
